# revision 14
# baseline (speedup 1.0000x reference)
"""BigVGAN 24k forward on 8 Trainium2 NeuronCores (Bass/Tile).

Sharding: 8 cores = batch(4) x sequence-halves(2); each core gets 280 mel
frames (256 valid + 24 halo), emits 71680 samples, host crops+stitches.

Layout: time-folded [C*F, W] (stage3 C=32 -> F=4) so all 128 partitions are
busy. Dense convs = one matmul per tap with block-diagonal lhsT (full 128
contraction). FIR up2/down2 = scaled-identity matmuls over a polyphase
decomposition (phases stored separately, never interleaved). SnakeBeta via
x + c1 - c1*sin(2a x + pi/2). Fold-boundary margins kept coherent by small
SBUF-to-SBUF fixup DMAs; conv bias applied as an extra rank-1 matmul tap.
"""
import math
import os
import sys

import numpy as np

sys.path.insert(0, '/opt/trn_rl_repo')

KDBG = int(os.environ.get('KDBG', '0'))  # >0: dump intermediate, truncate program

import ml_dtypes  # noqa: E402

BF16 = ml_dtypes.bfloat16

MEL = 100
UPS_R = [8, 8, 2, 2]
RK = [3, 7, 11]
RD = [1, 3, 5]

HALO = 24
TMEL = 280
MARG = 32
W = 17920
WS = W + 2 * MARG          # 17984
W0 = 2240
WS0 = W0 + 2 * MARG        # 2304
NT = 512
PM = 8                     # mel / pre-conv block margin
BW0 = TMEL + 2 * PM        # 296, pre-conv block width


def fir12():
    K = 12
    half = K // 2
    cutoff, half_width = 0.25, 0.3
    A = 2.285 * (half - 1) * math.pi * (4.0 * half_width) + 7.95
    beta = 0.1102 * (A - 8.7) if A > 50.0 else (
        0.5842 * (A - 21.0) ** 0.4 + 0.07886 * (A - 21.0) if A >= 21.0 else 0.0)
    t = np.arange(-half, half) + 0.5
    f = 2.0 * cutoff * np.kaiser(K, beta) * np.sinc(2.0 * cutoff * t)
    return (f / f.sum()).astype(np.float64)


FILT = fir12()
CE = np.array([2.0 * FILT[11 - 2 * j] for j in range(6)])  # ye: x[t-3+j]
CO = np.array([2.0 * FILT[10 - 2 * j] for j in range(6)])  # yo: x[t-2+j]
DE = np.array([FILT[2 * j + 1] for j in range(6)])         # ze[t-2+j]
DO = np.array([FILT[2 * j] for j in range(6)])             # zo[t-3+j]


def upsample_phase_taps(wt, u):
    """wt [Cin, Cout, K] (torch ConvTranspose1d). Per phase p: list of
    (lhsT [Cin, Cout], in_offset)."""
    Cin, Cout, K = wt.shape
    pad = K - 1 - (K - u) // 2
    Wm = np.flip(wt, -1).transpose(1, 0, 2)
    phases = []
    for p in range(u):
        taps = []
        for k in range(K):
            num = p + k - pad
            if num % u == 0:
                taps.append((Wm[:, :, k].T.copy(), num // u))
        phases.append(taps)
    return phases


class Blob:
    def __init__(self, dtype):
        self.cols = 0
        self.parts = []
        self.offs = {}
        self.dtype = dtype

    def add(self, name, arr, row0=0):
        arr = np.asarray(arr, np.float64)
        P, c = arr.shape
        a = np.zeros((128, c), np.float64)
        a[row0:row0 + P] = arr
        self.offs[name] = (self.cols, c)
        self.parts.append(a)
        self.cols += c

    def finalize(self):
        a = np.concatenate(self.parts, axis=1)
        return np.ascontiguousarray(a).astype(self.dtype)


def blockdiag(mat, C, F):
    out = np.zeros((C * F, C * F))
    for f in range(F):
        out[f * C:(f + 1) * C, f * C:(f + 1) * C] = mat
    return out


STAGES = [dict(C=256, F=1), dict(C=128, F=1), dict(C=64, F=2), dict(C=32, F=4)]


def prep_weights(params):
    wb = Blob(BF16)
    tb = Blob(np.float32)
    upoffs = []

    for nm, coefs in (('ce', CE), ('co', CO), ('de', DE), ('do', DO)):
        for j in range(6):
            wb.add(f'fir_{nm}{j}', np.eye(128) * coefs[j])
    tb.add('halfpi', np.full((128, 1), math.pi / 2))

    P = params

    def snake_cols(name, la, lb, F):
        a = np.exp(np.asarray(la, np.float64))
        b = np.exp(np.asarray(lb, np.float64))
        c1 = 1.0 / (2.0 * (b + 1e-9))
        tb.add(name + '_2a', np.tile(2.0 * a, F)[:, None])
        tb.add(name + '_c1', np.tile(c1, F)[:, None])
        tb.add(name + '_nc1', np.tile(-c1, F)[:, None])

    pw = np.asarray(P['pre_w'], np.float64)
    for ob in range(4):
        for t in range(7):
            wb.add(f'pre_{ob}_{t}', pw[ob * 128:(ob + 1) * 128, :, t].T)
        wb.add(f'pre_b{ob}',
               np.asarray(P['pre_b'], np.float64)[None, ob * 128:(ob + 1) * 128])

    for i in range(4):
        wt, b = P['ups'][i]
        wt = np.asarray(wt, np.float64)
        b = np.asarray(b, np.float64)
        phases = upsample_phase_taps(wt, UPS_R[i])
        upoffs.append([[off for (_, off) in taps] for taps in phases])
        Cin, Cout, _ = wt.shape
        n_ib, n_ob = (Cin + 127) // 128, (Cout + 127) // 128
        Cob = min(Cout, 128)
        for p, taps in enumerate(phases):
            for ti, (mat, _) in enumerate(taps):
                for ib in range(n_ib):
                    for ob in range(n_ob):
                        wb.add(f'up{i}_p{p}t{ti}i{ib}o{ob}',
                               mat[ib * 128:(ib + 1) * 128,
                                   ob * Cob:(ob + 1) * Cob])
                        if i == 3:
                            # partition-shifted copy for fold-1 source reads
                            wb.add(f'up{i}s_p{p}t{ti}i{ib}o{ob}',
                                   mat[ib * 128:(ib + 1) * 128,
                                       ob * Cob:(ob + 1) * Cob], row0=64)
        for ob in range(n_ob):
            wb.add(f'up{i}_b{ob}', b[None, ob * Cob:(ob + 1) * Cob])

    for i in range(4):
        C, F = STAGES[i]['C'], STAGES[i]['F']
        nblk = 2 if C == 256 else 1
        Cb = min(C, 128)
        for j in range(3):
            blk = P['res'][i][j]
            k = RK[j]
            for s in range(3):
                for cname in ('c1', 'c2'):
                    w_, b_ = blk[cname][s]
                    w_ = np.asarray(w_, np.float64)
                    b_ = np.asarray(b_, np.float64)
                    pref = f's{i}b{j}l{s}{cname}'
                    if nblk == 1:
                        for t in range(k):
                            wb.add(f'{pref}_t{t}', blockdiag(w_[:, :, t].T, C, F))
                        wb.add(f'{pref}_b0', np.tile(b_, F)[None, :])
                    else:
                        for t in range(k):
                            for ib in range(2):
                                for ob in range(2):
                                    wb.add(f'{pref}_t{t}i{ib}o{ob}',
                                           w_[ob * 128:(ob + 1) * 128,
                                              ib * 128:(ib + 1) * 128, t].T)
                        for ob in range(2):
                            wb.add(f'{pref}_b{ob}',
                                   b_[None, ob * 128:(ob + 1) * 128])
                for a in range(2):
                    la, lb = blk['act'][2 * s + a]
                    la = np.asarray(la, np.float64)
                    lb = np.asarray(lb, np.float64)
                    for ob in range(nblk):
                        snake_cols(f's{i}b{j}l{s}a{a}o{ob}',
                                   la[ob * Cb:(ob + 1) * Cb],
                                   lb[ob * Cb:(ob + 1) * Cb], F)

    snake_cols('post_a', P['post_a'], P['post_b'], 4)
    pow_ = np.asarray(P['post_w'], np.float64)
    for t in range(7):
        m = np.zeros((128, 4))
        for gidx in range(4):
            m[gidx * 32:(gidx + 1) * 32, gidx] = pow_[0, :, t]
        wb.add(f'post_t{t}', m)

    return wb, tb, upoffs


# ---------------------------------------------------------------------------

def build_program(wb, tb, upoffs):
    import concourse.mybir as mybir
    import concourse.tile as tile
    from concourse import bacc

    F32 = mybir.dt.float32
    B16 = mybir.dt.bfloat16
    AF = mybir.ActivationFunctionType
    ADD = mybir.AluOpType.add

    nc = bacc.Bacc("TRN2", target_bir_lowering=False, debug=False)

    xin = nc.declare_dram_parameter("xin", [128, TMEL], B16, isOutput=False)
    wbuf = nc.declare_dram_parameter("wbuf", [128, wb.cols], B16, isOutput=False)
    tbuf = nc.declare_dram_parameter("tbuf", [128, tb.cols], F32, isOutput=False)
    out = nc.declare_dram_parameter("out", [4, W], F32, isOutput=True)
    dbg = nc.declare_dram_parameter("dbg", [128, WS], F32, isOutput=True) \
        if KDBG else None
    xstage = nc.dram_tensor("xstage", [128, WS], B16)
    xsd = nc.dram_tensor("xsd", [128, WS], B16)

    class Stop(Exception):
        pass

    WPH = MARG + W // 2 + 24

    with tile.TileContext(nc) as tc:
        with tc.tile_pool(name="main", bufs=1) as main, \
             tc.tile_pool(name="wt", bufs=2) as wtp, \
             tc.tile_pool(name="tmp", bufs=4) as tmp, \
             tc.tile_pool(name="ps", bufs=6, space="PSUM") as psp:

            xb = main.tile([128, WS], B16, tag="xb")
            t1 = main.tile([128, WS], B16, tag="t1")
            t2 = main.tile([128, WS], B16, tag="t2")
            ph0 = main.tile([128, WPH], B16, tag="ph0")
            ph1 = main.tile([128, WPH], B16, tag="ph1")
            firw = main.tile([128, 24 * 128], B16, tag="firw")
            tabs = main.tile([128, tb.cols], F32, tag="tabs")
            ones = main.tile([1, NT], B16, tag="ones")

            nc.sync.dma_start(out=firw[:], in_=wbuf[:, 0:24 * 128])
            nc.sync.dma_start(out=tabs[:], in_=tbuf[:])
            nc.vector.memset(ones[:], 1.0)

            def fir(nm, j):
                o, _ = wb.offs[f'fir_{nm}{j}']
                return firw[:, o:o + 128]

            def tabcol(name):
                o, _ = tb.offs[name]
                return tabs[:, o:o + 1]

            def wtile(names):
                o0 = wb.offs[names[0]][0]
                oe, ce = wb.offs[names[-1]]
                cols = oe + ce - o0
                t = wtp.tile([128, 6400], B16, tag="wt")
                nc.sync.dma_start(out=t[:, :cols], in_=wbuf[:, o0:o0 + cols])
                return {n: t[:, wb.offs[n][0] - o0:
                             wb.offs[n][0] - o0 + wb.offs[n][1]]
                        for n in names}

            def T(lh, buf, p0, P, base, stride=1):
                if stride == 1:
                    return (lh, lambda t0, N, b=base: buf[p0:p0 + P, b + t0:
                                                          b + t0 + N])
                return (lh, lambda t0, N, b=base, s=stride:
                        buf[p0:p0 + P, b + t0 * s:b + (t0 + N) * s:s])

            def BT(bias_ap):
                return (bias_ap[0:1, :], lambda t0, N: ones[0:1, 0:N])

            def sweep(dst, taps, Wv, M=128, mode='copy', snakev=None):
                """dst: fn(t0, N) -> AP. taps: [(lhsT, rhs_fn)]."""
                ntap = len(taps)
                for t0 in range(0, Wv, NT):
                    N = min(NT, Wv - t0)
                    ps = psp.tile([128, NT], F32, tag="ps")
                    for idx, (lh, rf) in enumerate(taps):
                        nc.tensor.matmul(ps[0:M, 0:N], lh, rf(t0, N),
                                         start=(idx == 0), stop=(idx == ntap - 1))
                    dsl = dst(t0, N)
                    if mode == 'snake':
                        v2a, vc1, vnc1 = snakev
                        s_t = tmp.tile([128, NT], F32, tag="s_t")
                        u_t = tmp.tile([128, NT], F32, tag="u_t")
                        nc.scalar.activation(s_t[0:M, 0:N], ps[0:M, 0:N], AF.Sin,
                                             bias=tabcol('halfpi'), scale=v2a)
                        nc.scalar.activation(u_t[0:M, 0:N], ps[0:M, 0:N],
                                             AF.Identity, bias=vc1)
                        nc.vector.tensor_scalar_mul(s_t[0:M, 0:N],
                                                    s_t[0:M, 0:N], vnc1)
                        nc.vector.tensor_tensor(dsl, u_t[0:M, 0:N],
                                                s_t[0:M, 0:N], ADD)
                    elif mode == 'accum':
                        nc.vector.tensor_tensor(dsl, dsl, ps[0:M, 0:N], ADD)
                    elif mode == 'tanh':
                        f_t = tmp.tile([4, NT], F32, tag="f_t")
                        nc.scalar.activation(f_t[0:M, 0:N], ps[0:M, 0:N], AF.Tanh)
                        nc.sync.dma_start(out=dsl, in_=f_t[0:M, 0:N])
                    else:
                        nc.scalar.activation(dsl, ps[0:M, 0:N], AF.Identity)

            def dstf(buf, p0, M, base, stride=1):
                if stride == 1:
                    return lambda t0, N, b=base: buf[p0:p0 + M, b + t0:b + t0 + N]
                return lambda t0, N, b=base, s=stride: \
                    buf[p0:p0 + M, b + t0 * s:b + (t0 + N) * s:s]

            def fixups(buf, C, F, Wv, edge, col0=0):
                for f in range(1, F):
                    nc.sync.dma_start(
                        out=buf[f * C:(f + 1) * C, col0:col0 + MARG],
                        in_=buf[(f - 1) * C:f * C, col0 + Wv:col0 + Wv + MARG])
                    nc.sync.dma_start(
                        out=buf[(f - 1) * C:f * C,
                                col0 + MARG + Wv:col0 + 2 * MARG + Wv],
                        in_=buf[f * C:(f + 1) * C, col0 + MARG:col0 + 2 * MARG])
                lo = buf[0:C, col0:col0 + MARG]
                hi = buf[(F - 1) * C:F * C,
                         col0 + MARG + Wv:col0 + 2 * MARG + Wv]
                if edge == 'zero':
                    nc.vector.memset(lo, 0.0)
                    nc.vector.memset(hi, 0.0)
                else:
                    nc.vector.tensor_copy(
                        lo, buf[0:C, col0 + MARG:col0 + MARG + 1]
                        .broadcast_to((C, MARG)))
                    nc.vector.tensor_copy(
                        hi, buf[(F - 1) * C:F * C,
                                col0 + MARG + Wv - 1:col0 + MARG + Wv]
                        .broadcast_to((C, MARG)))

            def act1d(src, dst, C, F, pref, col0=0, Wv=W):
                snakev = (tabcol(pref + '_2a'), tabcol(pref + '_c1'),
                          tabcol(pref + '_nc1'))
                half = (Wv + 1) // 2
                for h0 in range(0, Wv, half):
                    h1 = min(h0 + half, Wv)
                    hw = h1 - h0 + 16
                    base = col0 + MARG + h0 - 8
                    sweep(dstf(ph0, 0, 128, MARG),
                          [T(fir('ce', j), src, 0, 128, base + j - 3)
                           for j in range(6)], hw, mode='snake', snakev=snakev)
                    sweep(dstf(ph1, 0, 128, MARG),
                          [T(fir('co', j), src, 0, 128, base + j - 2)
                           for j in range(6)], hw, mode='snake', snakev=snakev)
                    if h0 == 0:
                        zc = ph0[0:C, MARG + 8:MARG + 9]
                        nc.vector.tensor_copy(ph0[0:C, MARG:MARG + 8],
                                              zc.broadcast_to((C, 8)))
                        nc.vector.tensor_copy(ph1[0:C, MARG:MARG + 8],
                                              zc.broadcast_to((C, 8)))
                    if h1 == Wv:
                        zc = ph1[(F - 1) * C:F * C, MARG + hw - 9:MARG + hw - 8]
                        nc.vector.tensor_copy(
                            ph0[(F - 1) * C:F * C, MARG + hw - 8:MARG + hw],
                            zc.broadcast_to((C, 8)))
                        nc.vector.tensor_copy(
                            ph1[(F - 1) * C:F * C, MARG + hw - 8:MARG + hw],
                            zc.broadcast_to((C, 8)))
                    taps = ([T(fir('de', j), ph0, 0, 128, MARG + 8 + j - 2)
                             for j in range(6)] +
                            [T(fir('do', j), ph1, 0, 128, MARG + 8 + j - 3)
                             for j in range(6)])
                    sweep(dstf(dst, 0, 128, col0 + MARG + h0), taps, h1 - h0)

            def conv(src, dst, pref, k, dil, accum=False, col0=0, Wv=W,
                     nblk=1, src_col0=None):
                sc = col0 if src_col0 is None else src_col0
                if nblk == 1:
                    names = [f'{pref}_t{t}' for t in range(k)] + [f'{pref}_b0']
                    mats = wtile(names)
                    taps = [T(mats[f'{pref}_t{t}'], src, 0, 128,
                              sc + MARG + (t - (k - 1) // 2) * dil)
                            for t in range(k)]
                    taps.append(BT(mats[f'{pref}_b0']))
                    sweep(dstf(dst, 0, 128, col0 + MARG), taps, Wv,
                          mode='accum' if accum else 'copy')
                else:
                    names = ([f'{pref}_t{t}i{ib}o{ob}' for t in range(k)
                              for ib in range(2) for ob in range(2)] +
                             [f'{pref}_b0', f'{pref}_b1'])
                    mats = wtile(names)
                    for ob in range(2):
                        taps = []
                        for t in range(k):
                            for ib in range(2):
                                taps.append(T(mats[f'{pref}_t{t}i{ib}o{ob}'],
                                              src, 0, 128,
                                              ib * WS0 + MARG +
                                              (t - (k - 1) // 2) * dil))
                        taps.append(BT(mats[f'{pref}_b{ob}']))
                        sweep(dstf(dst, 0, 128, ob * WS0 + MARG), taps, Wv,
                              mode='accum' if accum else 'copy')

            # ------------------------- program -------------------------
            def ck(n, buf, cols=WS):
                """Debug checkpoint: dump buf and signal stop when KDBG==n."""
                if KDBG == n:
                    nc.gpsimd.dma_start(out=dbg[:, 0:cols], in_=buf[:, 0:cols])
                return KDBG == n

            def program():
                program_body()

            def program_body():
                emit_all()

            def emit_all():
                nc.vector.memset(t1[:, :], 0.0)
                nc.vector.memset(t2[:, :], 0.0)
                nc.vector.memset(xb[:, 0:2 * WS0], 0.0)
                nc.vector.memset(ph0[:, 0:BW0], 0.0)
                nc.gpsimd.dma_start(out=ph0[:, PM:PM + TMEL], in_=xin[:])
                emit_pre()
                if ck(1, t1):
                    return
                emit_up0()
                if ck(2, xb):
                    return
                if emit_stage0():
                    return
                if ck(3, t1):
                    return
                emit_up1()
                if ck(4, xb):
                    return
                for i in (1, 2, 3):
                    if emit_stage(i):
                        return
                    if ck(4 + i, t1):
                        return
                emit_post()

            def emit_pre():
                for ob in range(4):
                    names = ([f'pre_{ob}_{t}' for t in range(7)] +
                             [f'pre_b{ob}'])
                    mats = wtile(names)
                    taps = [T(mats[f'pre_{ob}_{t}'], ph0, 0, 128, PM + t - 3)
                            for t in range(7)]
                    taps.append(BT(mats[f'pre_b{ob}']))
                    sweep(dstf(t1, 0, 128, ob * BW0 + PM), taps, TMEL)

            def emit_up0():
                # t1 4-blocks -> xb stage0 2 col-blocks, stride 8
                for p in range(8):
                    names = [f'up0_p{p}t{ti}i{ib}o{ob}' for ti in range(2)
                             for ib in range(4) for ob in range(2)]
                    mats = wtile(names)
                    bmats = wtile(['up0_b0', 'up0_b1'])
                    for ob in range(2):
                        taps = []
                        for ti in range(2):
                            off = upoffs[0][p][ti]
                            for ib in range(4):
                                taps.append(T(mats[f'up0_p{p}t{ti}i{ib}o{ob}'],
                                              t1, 0, 128, ib * BW0 + PM + off))
                        taps.append(BT(bmats[f'up0_b{ob}']))
                        sweep(dstf(xb, 0, 128, ob * WS0 + MARG + p, stride=8),
                              taps, TMEL)
                for ob in range(2):
                    fixups(xb, 128, 1, W0, 'repl', col0=ob * WS0)
                nc.sync.dma_start(out=xstage[:, 0:2 * WS0],
                                  in_=xb[:, 0:2 * WS0])

            def xs_update(j, width, scale_third):
                if j == 0:
                    nc.sync.dma_start(out=xsd[:, 0:width], in_=xb[:, 0:width])
                else:
                    nc.sync.dma_start(out=t1[:, 0:width], in_=xsd[:, 0:width])
                    nc.vector.tensor_tensor(t1[:, 0:width], t1[:, 0:width],
                                            xb[:, 0:width], ADD)
                    if scale_third:
                        nc.vector.tensor_scalar_mul(t1[:, 0:width],
                                                    t1[:, 0:width], 1.0 / 3.0)
                    else:
                        nc.sync.dma_start(out=xsd[:, 0:width],
                                          in_=t1[:, 0:width])

            def emit_stage0():
                for j in range(3):
                    k = RK[j]
                    if j > 0:
                        nc.sync.dma_start(out=xb[:, 0:2 * WS0],
                                          in_=xstage[:, 0:2 * WS0])
                    for ob in range(2):
                        fixups(xb, 128, 1, W0, 'repl', col0=ob * WS0)
                    for s in range(3):
                        for ob in range(2):
                            act1d(xb, t1, 128, 1, f's0b{j}l{s}a0o{ob}',
                                  col0=ob * WS0, Wv=W0)
                            fixups(t1, 128, 1, W0, 'zero', col0=ob * WS0)
                        if j == 0 and s == 0 and ck(21, t1, 2 * WS0):
                            return True
                        conv(t1, t2, f's0b{j}l{s}c1', k, RD[s], Wv=W0, nblk=2)
                        for ob in range(2):
                            fixups(t2, 128, 1, W0, 'repl', col0=ob * WS0)
                        if j == 0 and s == 0 and ck(22, t2, 2 * WS0):
                            return True
                        for ob in range(2):
                            act1d(t2, t1, 128, 1, f's0b{j}l{s}a1o{ob}',
                                  col0=ob * WS0, Wv=W0)
                            fixups(t1, 128, 1, W0, 'zero', col0=ob * WS0)
                        conv(t1, xb, f's0b{j}l{s}c2', k, 1, accum=True, Wv=W0,
                             nblk=2)
                        for ob in range(2):
                            fixups(xb, 128, 1, W0, 'repl', col0=ob * WS0)
                        if j == 0 and s == 0 and ck(23, xb, 2 * WS0):
                            return True
                    xs_update(j, 2 * WS0, j == 2)
                for ob in range(2):
                    fixups(t1, 128, 1, W0, 'zero', col0=ob * WS0)
                return False

            def emit_up1():
                for p in range(8):
                    names = [f'up1_p{p}t{ti}i{ib}o0' for ti in range(2)
                             for ib in range(2)]
                    mats = wtile(names)
                    bmats = wtile(['up1_b0'])
                    taps = []
                    for ti in range(2):
                        off = upoffs[1][p][ti]
                        for ib in range(2):
                            taps.append(T(mats[f'up1_p{p}t{ti}i{ib}o0'],
                                          t1, 0, 128, ib * WS0 + MARG + off))
                    taps.append(BT(bmats['up1_b0']))
                    sweep(dstf(xb, 0, 128, MARG + p, stride=8), taps, W0)
                fixups(xb, 128, 1, W, 'repl')
                nc.sync.dma_start(out=xstage[:], in_=xb[:])

            def emit_stage(i):
                C, F = STAGES[i]['C'], STAGES[i]['F']
                for j in range(3):
                    k = RK[j]
                    if j > 0:
                        nc.sync.dma_start(out=xb[:], in_=xstage[:])
                    fixups(xb, C, F, W, 'repl')
                    for s in range(3):
                        act1d(xb, t1, C, F, f's{i}b{j}l{s}a0o0')
                        fixups(t1, C, F, W, 'zero')
                        if i == 1 and j == 0 and s == 0 and ck(31, t1):
                            return True
                        conv(t1, t2, f's{i}b{j}l{s}c1', k, RD[s])
                        fixups(t2, C, F, W, 'repl')
                        if i == 1 and j == 0 and s == 0 and ck(32, t2):
                            return True
                        act1d(t2, t1, C, F, f's{i}b{j}l{s}a1o0')
                        fixups(t1, C, F, W, 'zero')
                        conv(t1, xb, f's{i}b{j}l{s}c2', k, 1, accum=True)
                        fixups(xb, C, F, W, 'repl')
                        if i == 1 and j == 0 and s == 0 and ck(33, xb):
                            return True
                    xs_update(j, WS, j == 2)
                if i < 3:
                    # upsample by 2 into next stage's fold layout
                    fixups(t1, C, F, W, 'zero')
                    Cn = STAGES[i + 1]['C']
                    for g in range(STAGES[i + 1]['F']):
                        fi, loc = divmod(g, 2) if i == 2 else (0, g)
                        p0 = fi * C
                        sfx = 's' if fi == 1 else ''
                        for p in range(2):
                            names = [f'up{i + 1}{sfx}_p{p}t{ti}i0o0'
                                     for ti in range(2)]
                            mats = wtile(names)
                            bmats = wtile([f'up{i + 1}_b0'])
                            taps = []
                            for ti in range(2):
                                off = upoffs[i + 1][p][ti]
                                taps.append(
                                    (mats[f'up{i + 1}{sfx}_p{p}t{ti}i0o0']
                                     [p0:p0 + C, :],
                                     (lambda t0, N,
                                      b=MARG + loc * (W // 2) + off, q0=p0:
                                      t1[q0:q0 + C, b + t0:b + t0 + N])))
                            taps.append(BT(bmats[f'up{i + 1}_b0']))
                            sweep(dstf(xb, g * Cn, Cn, MARG + p, stride=2),
                                  taps, W // 2, M=Cn)
                    fixups(xb, Cn, STAGES[i + 1]['F'], W, 'repl')
                    nc.sync.dma_start(out=xstage[:], in_=xb[:])
                return False

            def emit_post():
                fixups(t1, 32, 4, W, 'repl')
                act1d(t1, t2, 32, 4, 'post_a')
                fixups(t2, 32, 4, W, 'zero')
                if ck(8, t2):
                    return
                names = [f'post_t{t}' for t in range(7)]
                mats = wtile(names)
                taps = [T(mats[f'post_t{t}'], t2, 0, 128, MARG + t - 3)
                        for t in range(7)]
                sweep(lambda t0, N: out[0:4, t0:t0 + N], taps, W, M=4,
                      mode='tanh')

            program()

    nc.compile()
    return nc


# ---------------------------------------------------------------------------

_CACHED = {}


def _params_np(params):
    import jax
    return jax.tree.map(lambda a: np.asarray(a), params,
                        is_leaf=lambda a: hasattr(a, 'shape'))


def kernel(x, params):
    from concourse.bass_utils import run_bass_kernel_spmd

    x = np.asarray(x, np.float32)
    params = _params_np(params)
    wb, tb, upoffs = prep_weights(params)
    if 'nc' not in _CACHED:
        _CACHED['nc'] = build_program(wb, tb, upoffs)
    nc = _CACHED['nc']

    wblob = wb.finalize()
    tblob = tb.finalize()
    in_maps = []
    for c in range(8):
        b, h = divmod(c, 2)
        mel = x[b, :, :TMEL] if h == 0 else x[b, :, 512 - TMEL:]
        xi = np.zeros((128, TMEL), np.float32)
        xi[:MEL] = mel
        in_maps.append({"xin": xi.astype(BF16), "wbuf": wblob, "tbuf": tblob})

    res = run_bass_kernel_spmd(nc, in_maps, list(range(8)))
    _CACHED['last_res'] = res

    outp = np.zeros((4, 1, 131072), np.float32)
    for c in range(8):
        b, h = divmod(c, 2)
        audio = np.asarray(res.results[c]["out"], np.float32).reshape(-1)
        if h == 0:
            outp[b, 0, :65536] = audio[:65536]
        else:
            outp[b, 0, 65536:] = audio[HALO * 256:]
    return outp
